# revision 1
# baseline (speedup 1.0000x reference)
"""Trainium2 Bass kernel for Conv2dFusion (outer-product -> 3x(conv3x3+gelu+maxpool2)).

Sharding: data-parallel over batch; core c processes batch c (64 tokens).
conv1 uses the rank-1 structure of the fused outer-product image:
    y1[co,i,j] = sum_dy u[i+dy] * V[co,dy,j],  V[co,dy,j] = sum_dx w0[co,0,dy,dx] v[j+dx]
with bf16 hi/lo splits on u, v, w0 and V for near-fp32 accuracy; conv1 bias is
folded in as a 13th all-ones lhsT row. conv2/conv3 use (dy,ci)-stacked
contraction with dx realized as shifted rhs reads, and even/odd output chunks
run concurrently in separate PE column groups (tile_position).
"""

from contextlib import ExitStack

import numpy as np
import ml_dtypes

import concourse.bass as bass
import concourse.bacc as bacc
import concourse.tile as tile
import concourse.mybir as mybir
from concourse.bass_utils import run_bass_kernel_spmd

BF16 = mybir.dt.bfloat16
F32 = mybir.dt.float32
GELU = mybir.ActivationFunctionType.Gelu

NCORES = 8


def _split_bf16(x):
    hi = x.astype(ml_dtypes.bfloat16)
    lo = (x - hi.astype(np.float32)).astype(ml_dtypes.bfloat16)
    return hi, lo


def _ap(base, ap, off=0):
    """New AP over base's tensor with explicit [step, count] dims (element units).

    For SBUF APs dim0 is the partition dim; to start at partition p add
    p * base.ap[0][0] to `off` (partition pitch is encoded in the dim0 step).
    """
    return bass.AP(tensor=base.tensor, offset=base.offset + off, ap=ap)


def build_kernel(T):
    """T = tokens per core."""
    nc = bacc.Bacc("TRN2", target_bir_lowering=False, debug=False)

    ushift = nc.dram_tensor("ushift", [13, 64 * 126 + 8], BF16, kind="ExternalInput")
    vshift = nc.dram_tensor("vshift", [12, 64 * 126], BF16, kind="ExternalInput")
    w0s = nc.dram_tensor("w0s", [12, 96], BF16, kind="ExternalInput")
    w1s = nc.dram_tensor("w1s", [96, 192], BF16, kind="ExternalInput")
    w2a = nc.dram_tensor("w2a", [128, 192], BF16, kind="ExternalInput")
    w2b = nc.dram_tensor("w2b", [64, 192], BF16, kind="ExternalInput")
    b0bc = nc.dram_tensor("b0bc", [1, 4096], BF16, kind="ExternalInput")
    b1c = nc.dram_tensor("b1c", [128, 1], F32, kind="ExternalInput")
    b2c = nc.dram_tensor("b2c", [128, 1], F32, kind="ExternalInput")
    out = nc.dram_tensor("out", [T, 12544], F32, kind="ExternalOutput")

    with tile.TileContext(nc) as tc, ExitStack() as ctx:
        consts = ctx.enter_context(tc.tile_pool(name="consts", bufs=1))
        ps = ctx.enter_context(tc.tile_pool(name="ps", bufs=4, space="PSUM"))
        sb2 = ctx.enter_context(tc.tile_pool(name="sb2", bufs=2))
        sb3 = ctx.enter_context(tc.tile_pool(name="sb3", bufs=3))
        dr = ctx.enter_context(tc.tile_pool(name="dr", bufs=2, space="DRAM"))

        c_ushift = consts.tile([13, 64 * 126 + 8], BF16)
        c_vshift = consts.tile([12, 64 * 126], BF16)
        c_w0s = consts.tile([12, 96], BF16)
        c_w1s = consts.tile([96, 192], BF16)
        c_w2a = consts.tile([128, 192], BF16)
        c_w2b = consts.tile([64, 192], BF16)
        c_b1 = consts.tile([128, 1], F32)
        c_b2 = consts.tile([128, 1], F32)
        nc.sync.dma_start(out=c_ushift[:], in_=ushift[:])
        nc.sync.dma_start(out=c_vshift[:], in_=vshift[:])
        nc.sync.dma_start(out=c_w0s[:], in_=w0s[:])
        nc.sync.dma_start(out=c_w1s[:], in_=w1s[:])
        nc.sync.dma_start(out=c_w2a[:], in_=w2a[:])
        nc.sync.dma_start(out=c_w2b[:], in_=w2b[:])
        nc.sync.dma_start(out=c_b1[:], in_=b1c[:])
        nc.sync.dma_start(out=c_b2[:], in_=b2c[:])

        upstep = c_ushift[:].ap[0][0]

        ngroups = (T + 3) // 4
        for g in range(ngroups):
            tok0 = g * 4
            # ---- stage A: V[(dy,co), (t4,par,j2)] = sum_dx w0 * v(shifted) ----
            psV = ps.tile([96, 504], F32, tag="ps")
            nc.tensor.matmul(
                psV[:, :], lhsT=c_w0s[:], rhs=c_vshift[:, g * 504:(g + 1) * 504],
                start=True, stop=True)
            vhi = sb2.tile([96, 504], BF16, tag="vhi")
            vlo = sb2.tile([96, 504], BF16, tag="vlo")
            nc.vector.tensor_copy(out=vhi[:], in_=psV[:, :])
            nc.vector.tensor_sub(vlo[:], psV[:, :], vhi[:])
            # bounce V to DRAM reordered as [t4, dy, co, (par,j2)=126]
            tvhi = dr.tile([4, 3, 32, 126], BF16, tag="tvhi")
            tvlo = dr.tile([4, 3, 32, 126], BF16, tag="tvlo")
            for src, dst in ((vhi, tvhi), (vlo, tvlo)):
                sstep = src[:].ap[0][0]
                for dy in range(3):
                    nc.sync.dma_start(
                        out=_ap(dst[:], [[126, 32], [12096, 4], [1, 126]],
                                off=dy * 4032),
                        in_=_ap(src[:], [[sstep, 32], [1, 504]],
                                off=dy * 32 * sstep),
                    )

            for t4 in range(min(4, T - tok0)):
                t = tok0 + t4
                # ---- V3_4 rhs [13, 4104]: rows Vh*6+uh*3+dy, free co*128+par*64+j2
                v34 = sb3.tile([13, 4104], BF16, tag="v34")
                v34step = v34[:].ap[0][0]
                for vh, tv in ((0, tvhi), (1, tvlo)):
                    for uh in range(2):
                        for par in range(2):
                            nc.sync.dma_start(
                                out=_ap(v34[:], [[v34step, 3], [128, 32], [1, 63]],
                                        off=(vh * 6 + uh * 3) * v34step + par * 64),
                                in_=_ap(tv[:], [[4032, 3], [126, 32], [1, 63]],
                                        off=t4 * 12096 + par * 63),
                            )
                nc.sync.dma_start(out=v34[12:13, 0:4096], in_=b0bc[:])

                # ---- stage B: y1 quarters -> gelu -> g1 [128, 4096] bf16
                g1 = sb2.tile([128, 4096], BF16, tag="g1")
                lhsT_e = _ap(c_ushift[:], [[upstep, 13], [2, 64]], off=t * 126)
                lhsT_o = _ap(c_ushift[:], [[upstep, 13], [2, 64]], off=t * 126 + 1)
                for q in range(4):
                    pq = ps.tile([128, 1024], F32, tag="ps")
                    for half in range(2):  # chunk c = 2q+half covers co 4c..4c+3
                        c = 2 * q + half
                        rhs = v34[:, c * 512:(c + 1) * 512]
                        nc.tensor.matmul(pq[0:64, half * 512:(half + 1) * 512],
                                         lhsT=lhsT_e, rhs=rhs, start=True, stop=True,
                                         tile_position=(0, 0))
                        nc.tensor.matmul(pq[64:128, half * 512:(half + 1) * 512],
                                         lhsT=lhsT_o, rhs=rhs, start=True, stop=True,
                                         tile_position=(0, 64))
                    nc.scalar.activation(out=g1[:, q * 1024:(q + 1) * 1024],
                                         in_=pq[:, :], func=GELU)

                # ---- pool1: j-pairs (parity halves) then i-pairs (e/o halves)
                g1step = g1[:].ap[0][0]
                mj = sb2.tile([128, 2048], BF16, tag="mj")
                nc.vector.tensor_max(
                    mj[:],
                    _ap(g1[:], [[g1step, 128], [128, 32], [1, 64]]),
                    _ap(g1[:], [[g1step, 128], [128, 32], [1, 64]], off=64),
                )
                # ---- pool1 i-pairs: shift odd half down to partition 0 via DMA,
                # then max on gpsimd (tensor_tensor needs equal base partitions)
                mjstep = mj[:].ap[0][0]
                mjb = sb2.tile([64, 2048], BF16, tag="mjb")
                nc.sync.dma_start(
                    out=mjb[0:63, :],
                    in_=_ap(mj[:], [[mjstep, 63], [1, 2048]], off=64 * mjstep),
                )
                p1 = sb2.tile([63, 2048], BF16, tag="p1")
                nc.vector.tensor_max(p1[:], mj[0:63, :], mjb[0:63, :])

                # ---- bounce p1 -> tmp1[co, i2*63+j2]
                tmp1 = dr.tile([32, 3969], BF16, tag="tmp1")
                nc.sync.dma_start(
                    out=_ap(tmp1[:], [[63, 63], [3969, 32], [1, 63]]),
                    in_=_ap(p1[:], [[p1[:].ap[0][0], 63], [64, 32], [1, 63]]),
                )
                # ---- S2 [96, 4104]: row (dy,ci)[y*64+x] = tmp1[ci, (y+dy)*63+x]
                s2 = sb3.tile([96, 4104], BF16, tag="s2")
                s2step = s2[:].ap[0][0]
                for dy in range(3):
                    nc.sync.dma_start(
                        out=_ap(s2[:], [[s2step, 32], [64, 63 - dy], [1, 63]],
                                off=dy * 32 * s2step),
                        in_=_ap(tmp1[:], [[3969, 32], [63, 63 - dy], [1, 63]],
                                off=dy * 63),
                    )

                # ---- conv2: 8 chunks x 3 dx, col-tiled pairs, 2 psum slots
                g2 = sb2.tile([128, 2048], BF16, tag="g2")
                for s in range(2):
                    p2s = ps.tile([128, 1024], F32, tag="ps")
                    for k in range(4):
                        c = 4 * s + k
                        pos = 64 * (c % 2)
                        od = 512 * (k // 2)
                        for dx in range(3):
                            nc.tensor.matmul(
                                p2s[pos:pos + 64, od:od + 512],
                                lhsT=c_w1s[:, dx * 64:(dx + 1) * 64],
                                rhs=s2[:, c * 512 + dx: c * 512 + dx + 512],
                                start=(dx == 0), stop=(dx == 2),
                                tile_position=(0, pos))
                    nc.scalar.activation(out=g2[:, s * 1024:(s + 1) * 1024],
                                         in_=p2s[:, :], func=GELU, bias=c_b1[:])

                # ---- pool2: x-pairs then y-pairs
                g2step = g2[:].ap[0][0]
                mx2 = sb2.tile([128, 1024], BF16, tag="mx2")
                mx2step = mx2[:].ap[0][0]
                nc.vector.tensor_max(
                    _ap(mx2[:], [[mx2step, 128], [256, 4], [32, 8], [1, 31]]),
                    _ap(g2[:], [[g2step, 128], [512, 4], [64, 8], [2, 31]]),
                    _ap(g2[:], [[g2step, 128], [512, 4], [64, 8], [2, 31]], off=1),
                )
                p2 = sb2.tile([128, 512], BF16, tag="p2")
                p2step = p2[:].ap[0][0]
                nc.vector.tensor_max(
                    _ap(p2[:], [[p2step, 128], [128, 4], [32, 4], [1, 32]]),
                    _ap(mx2[:], [[mx2step, 128], [256, 4], [64, 4], [1, 32]]),
                    _ap(mx2[:], [[mx2step, 128], [256, 4], [64, 4], [1, 32]], off=32),
                )

                # ---- bounce p2 -> tmp3[co, y2*30+x2] (alloc 1024 with junk pad)
                tmp3 = dr.tile([64, 1024], BF16, tag="tmp3")
                for b in range(4):  # top half: y2 = 8b+q
                    nc.sync.dma_start(
                        out=_ap(tmp3[:], [[1024, 64], [30, 4], [1, 30]],
                                off=b * 240),
                        in_=_ap(p2[:], [[p2step, 64], [32, 4], [1, 30]],
                                off=b * 128),
                    )
                for b in range(4):  # bottom half: y2 = 8b+4+q (b=3: only q<2)
                    nq = 4 if b < 3 else 2
                    nc.sync.dma_start(
                        out=_ap(tmp3[:], [[1024, 64], [30, nq], [1, 30]],
                                off=b * 240 + 120),
                        in_=_ap(p2[:], [[p2step, 64], [32, nq], [1, 30]],
                                off=64 * p2step + b * 128),
                    )

                # ---- S3a [128, 968] rows (dy01, ci); S3b [64, 968] dy=2
                s3a = sb2.tile([128, 968], BF16, tag="s3a")
                s3b = sb2.tile([64, 968], BF16, tag="s3b")
                nc.sync.dma_start(
                    out=s3a[:, 0:962],
                    in_=_ap(tmp3[:], [[30, 2], [1024, 64], [1, 962]]),
                )
                nc.sync.dma_start(
                    out=s3b[:, 0:962],
                    in_=_ap(tmp3[:], [[1024, 64], [1, 962]], off=60),
                )

                # ---- conv3: 2 chunks (N=480) col-tiled, 6 accumulating MMs each
                p3s = ps.tile([128, 1024], F32, tag="ps")
                for c in range(2):
                    pos = 64 * c
                    for dx in range(3):
                        nc.tensor.matmul(
                            p3s[pos:pos + 64, 0:480],
                            lhsT=c_w2a[:, dx * 64:(dx + 1) * 64],
                            rhs=s3a[:, c * 480 + dx: c * 480 + dx + 480],
                            start=(dx == 0), stop=False, tile_position=(0, pos))
                    for dx in range(3):
                        nc.tensor.matmul(
                            p3s[pos:pos + 64, 0:480],
                            lhsT=c_w2b[:, dx * 64:(dx + 1) * 64],
                            rhs=s3b[:, c * 480 + dx: c * 480 + dx + 480],
                            start=False, stop=(dx == 2), tile_position=(0, pos))
                g3 = sb2.tile([128, 480], BF16, tag="g3")
                nc.scalar.activation(out=g3[:], in_=p3s[:, 0:480], func=GELU,
                                     bias=c_b2[:])

                # ---- pool3
                g3step = g3[:].ap[0][0]
                mx3 = sb2.tile([128, 256], BF16, tag="mx3")
                mx3step = mx3[:].ap[0][0]
                nc.vector.tensor_max(
                    _ap(mx3[:], [[mx3step, 128], [16, 16], [1, 15]]),
                    _ap(g3[:], [[g3step, 128], [30, 16], [2, 15]]),
                    _ap(g3[:], [[g3step, 128], [30, 16], [2, 15]], off=1),
                )
                p3 = sb2.tile([128, 128], F32, tag="p3")
                p3step = p3[:].ap[0][0]
                nc.vector.tensor_max(
                    _ap(p3[:], [[p3step, 128], [16, 8], [1, 16]]),
                    _ap(mx3[:], [[mx3step, 128], [32, 8], [1, 16]]),
                    _ap(mx3[:], [[mx3step, 128], [32, 8], [1, 16]], off=16),
                )

                # ---- output: out[t, co*196 + y2*14 + x2]
                nc.sync.dma_start(
                    out=_ap(out[:], [[196, 64], [14, 8], [1, 14]], off=t * 12544),
                    in_=_ap(p3[:], [[p3step, 64], [16, 8], [1, 14]]),
                )
                nc.sync.dma_start(
                    out=_ap(out[:], [[196, 64], [14, 6], [1, 14]],
                            off=t * 12544 + 112),
                    in_=_ap(p3[:], [[p3step, 64], [16, 6], [1, 14]],
                            off=64 * p3step),
                )

    nc.compile()
    return nc


def _host_prep(token_features, type_embedds, w0, b0, w1, b1, w2, b2):
    """Build per-core and shared device input arrays."""
    B, S, Dd = token_features.shape
    assert (B, S, Dd) == (8, 64, 128)
    shared = {}

    w0h, w0l = _split_bf16(w0.astype(np.float32))
    w0s = np.zeros((12, 96), dtype=ml_dtypes.bfloat16)
    for dx in range(3):
        for vh in range(2):
            for wh in range(2):
                r = dx * 4 + vh * 2 + wh
                src = (w0h, w0l)[wh][:, 0, :, dx]           # [co, dy]
                w0s[r, :] = np.transpose(src, (1, 0)).reshape(96)  # dy*32+co
    shared["w0s"] = w0s

    w1b = w1.astype(ml_dtypes.bfloat16)                     # [64, 32, 3, 3]
    w1s = np.zeros((96, 192), dtype=ml_dtypes.bfloat16)
    for dy in range(3):
        for dx in range(3):
            w1s[dy * 32:(dy + 1) * 32, dx * 64:(dx + 1) * 64] = w1b[:, :, dy, dx].T
    shared["w1s"] = w1s

    w2q = w2.astype(ml_dtypes.bfloat16)                     # [64, 64, 3, 3]
    w2a = np.zeros((128, 192), dtype=ml_dtypes.bfloat16)
    w2b_ = np.zeros((64, 192), dtype=ml_dtypes.bfloat16)
    for dy in range(2):
        for dx in range(3):
            w2a[dy * 64:(dy + 1) * 64, dx * 64:(dx + 1) * 64] = w2q[:, :, dy, dx].T
    for dx in range(3):
        w2b_[:, dx * 64:(dx + 1) * 64] = w2q[:, :, 2, dx].T
    shared["w2a"] = w2a
    shared["w2b"] = w2b_

    b0bc = np.zeros((1, 4096), dtype=ml_dtypes.bfloat16)
    for co in range(32):
        b0bc[0, co * 128:(co + 1) * 128] = np.float32(b0[co])
    shared["b0bc"] = b0bc
    shared["b1c"] = np.concatenate([b1, b1]).astype(np.float32).reshape(128, 1)
    shared["b2c"] = np.concatenate([b2, b2]).astype(np.float32).reshape(128, 1)

    per_core = []
    for c in range(B):
        u = type_embedds[c].astype(np.float32)              # [64, 128]
        v = token_features[c].astype(np.float32)
        uh, ul = _split_bf16(u)
        vh_, vl = _split_bf16(v)
        ush = np.zeros((13, 64 * 126 + 8), dtype=ml_dtypes.bfloat16)
        for Vh in range(2):
            for uhl in range(2):
                for dy in range(3):
                    r = Vh * 6 + uhl * 3 + dy
                    src = (uh, ul)[uhl]
                    ush[r, :64 * 126] = src[:, dy:dy + 126].reshape(-1)
        ush[12, :] = np.float32(1.0)
        vsh = np.zeros((12, 64 * 126), dtype=ml_dtypes.bfloat16)
        for dx in range(3):
            for vhl in range(2):
                for wh in range(2):
                    r = dx * 4 + vhl * 2 + wh
                    src = (vh_, vl)[vhl]
                    arr = np.zeros((64, 2, 63), dtype=ml_dtypes.bfloat16)
                    arr[:, 0, :] = src[:, dx:dx + 126:2][:, :63]   # par=0: j=2*j2
                    arr[:, 1, :] = src[:, dx + 1:dx + 127:2][:, :63]  # par=1
                    vsh[r, :] = arr.reshape(-1)
        per_core.append({"ushift": ush, "vshift": vsh})
    return shared, per_core


_CACHE = {}


def _get_kernel(T=64):
    if T not in _CACHE:
        _CACHE[T] = build_kernel(T)
    return _CACHE[T]


def kernel(token_features, type_embedds, w0, b0, w1, b1, w2, b2):
    token_features = np.asarray(token_features, dtype=np.float32)
    type_embedds = np.asarray(type_embedds, dtype=np.float32)
    shared, per_core = _host_prep(token_features, type_embedds,
                                  np.asarray(w0, dtype=np.float32),
                                  np.asarray(b0, dtype=np.float32),
                                  np.asarray(w1, dtype=np.float32),
                                  np.asarray(b1, dtype=np.float32),
                                  np.asarray(w2, dtype=np.float32),
                                  np.asarray(b2, dtype=np.float32))
    nc = _get_kernel(64)
    in_maps = [dict(shared, **per_core[c]) for c in range(NCORES)]
    res = run_bass_kernel_spmd(nc, in_maps, core_ids=list(range(NCORES)))
    outs = np.stack([res.results[c]["out"] for c in range(NCORES)])
    return outs.astype(np.float32)



# revision 10
# speedup vs baseline: 2.8964x; 2.8964x over previous
"""Trainium2 Bass kernel for Conv2dFusion (outer-product -> 3x(conv3x3+gelu+maxpool2)).

Sharding: data-parallel over batch; core c processes batch c (64 tokens).
conv1 uses the rank-1 structure of the fused outer-product image:
    y1[co,i,j] = sum_dy u[i+dy] * V[co,dy,j],  V[co,dy,j] = sum_dx w0[co,0,dy,dx] v[j+dx]
with bf16 hi/lo splits on u, v, w0 and V for near-fp32 accuracy; conv1 bias is
folded in as a 13th all-ones lhsT row. conv2/conv3 use (dy,ci)-stacked
contraction with dx realized as shifted rhs reads, and even/odd output chunks
run concurrently in separate PE column groups (tile_position).

Dispatch-cost optimizations (the end-to-end time through the axon tunnel is
dominated by host<->device transfer and jit rebuild, not device compute):
  - all inputs packed into one small bf16 blob per core; the shifted
    ushift/vshift layouts are built on-device with one-time SBUF DMAs
  - output is bf16 (the last pipeline stage is already bf16, so this is
    numerically free) which halves both the donated zero-buffer upload and
    the result download
  - jax persistent compilation cache avoids recompiling the wrapped NEFF
    executable on every run_bass_kernel_spmd call
"""

from contextlib import ExitStack

import numpy as np
import ml_dtypes

import jax

try:
    jax.config.update("jax_enable_compilation_cache", True)
    jax.config.update("jax_compilation_cache_dir", "/tmp/jax_bass_cache")
    jax.config.update("jax_persistent_cache_min_entry_size_bytes", -1)
    jax.config.update("jax_persistent_cache_min_compile_time_secs", 0)
except Exception:
    pass

import concourse.bass as bass
import concourse.bacc as bacc
import concourse.tile as tile
import concourse.mybir as mybir
from concourse.bass_utils import run_bass_kernel_spmd

BF16 = mybir.dt.bfloat16
F32 = mybir.dt.float32
GELU = mybir.ActivationFunctionType.Gelu

NCORES = 8

# blob layout (element offsets, bf16)
OFF_U = 0          # u_hi, u_lo           [2, 8192]
OFF_EO = 16384     # v even|odd hi, lo    [2, 8192]
OFF_W0S = 32768    # [12, 96]
OFF_W1S = 33920    # [96, 192]
OFF_W2A = 52352    # [128, 192]
OFF_W2B = 76928    # [64, 192]
OFF_B0 = 89216     # [1, 4096]
NBLOB = 93312


def _split_bf16(x):
    hi = x.astype(ml_dtypes.bfloat16)
    lo = (x - hi.astype(np.float32)).astype(ml_dtypes.bfloat16)
    return hi, lo


def _ap(base, ap, off=0):
    """New AP over base's tensor with explicit [step, count] dims (element units).

    For SBUF APs dim0 is the partition dim; to start at partition p add
    p * base.ap[0][0] to `off` (partition pitch is encoded in the dim0 step).
    """
    return bass.AP(tensor=base.tensor, offset=base.offset + off, ap=ap)


def build_kernel(T):
    """T = tokens per core."""
    nc = bacc.Bacc("TRN2", target_bir_lowering=False, debug=False)

    blob = nc.dram_tensor("blob", [1, NBLOB], BF16, kind="ExternalInput")
    bcomb = nc.dram_tensor("bcomb", [128, 2], F32, kind="ExternalInput")
    out = nc.dram_tensor("out", [T, 12544], BF16, kind="ExternalOutput")

    with tile.TileContext(nc) as tc, ExitStack() as ctx:
        consts = ctx.enter_context(tc.tile_pool(name="consts", bufs=1))
        ps = ctx.enter_context(tc.tile_pool(name="ps", bufs=4, space="PSUM"))
        sb2 = ctx.enter_context(tc.tile_pool(name="sb2", bufs=2))
        sb3 = ctx.enter_context(tc.tile_pool(name="sb3", bufs=3))
        dr = ctx.enter_context(tc.tile_pool(name="dr", bufs=2, space="DRAM"))

        c_ushift = consts.tile([13, 64 * 126 + 8], BF16)
        c_vshift = consts.tile([12, 64 * 126], BF16)
        c_w0s = consts.tile([12, 96], BF16)
        c_w1s = consts.tile([96, 192], BF16)
        c_w2a = consts.tile([128, 192], BF16)
        c_w2b = consts.tile([64, 192], BF16)
        c_b0bc = consts.tile([1, 4096], BF16)
        c_bb = consts.tile([128, 2], F32)

        nc.sync.dma_start(out=c_w0s[:], in_=_ap(blob[:], [[96, 12], [1, 96]], off=OFF_W0S))
        nc.sync.dma_start(out=c_w1s[:], in_=_ap(blob[:], [[192, 96], [1, 192]], off=OFF_W1S))
        nc.sync.dma_start(out=c_w2a[:], in_=_ap(blob[:], [[192, 128], [1, 192]], off=OFF_W2A))
        nc.sync.dma_start(out=c_w2b[:], in_=_ap(blob[:], [[192, 64], [1, 192]], off=OFF_W2B))
        nc.sync.dma_start(out=c_b0bc[:], in_=_ap(blob[:], [[4096, 1], [1, 4096]], off=OFF_B0))
        nc.sync.dma_start(out=c_bb[:], in_=bcomb[:])

        upstep = c_ushift[:].ap[0][0]
        vstep = c_vshift[:].ap[0][0]
        bbstep = c_bb[:].ap[0][0]

        # fill all 13 ushift rows with 1.0 first (row 12 stays the all-ones
        # bias row; rows 0-11 get overwritten below except the 8-col pad,
        # whose reads only feed discarded output rows). Must start at
        # partition 0: DVE ops can't base at partition 12.
        nc.vector.memset(_ap(c_ushift[:], [[upstep, 13], [1, 8072]]), 1.0)

        # ---- build ushift rows from the DRAM blob: row (Vh,uh,dy) =
        # u_{uh}[:, dy:dy+126] flattened. One DMA per (Vh,uh) covers the 3
        # dy rows; src iterates (dy: +1, t: +128, j: +1).
        for Vh in range(2):
            for uh_i in range(2):
                nc.sync.dma_start(
                    out=_ap(c_ushift[:], [[upstep, 3], [126, 64], [1, 126]],
                            off=(Vh * 6 + uh_i * 3) * upstep),
                    in_=_ap(blob[:], [[1, 3], [128, 64], [1, 126]],
                            off=OFF_U + uh_i * 8192),
                )
        # ---- build vshift rows from the blob's v even|odd region: row
        # (dx,vhl,wh) par-half j-deinterleave of v. One DMA per (dx,wh,par)
        # covers the vhl pair (partition rows r, r+2).
        PAR0 = [0, 64, 1]
        PAR1 = [64, 1, 65]
        for dx in range(3):
            for wh in range(2):
                for par, poff in ((0, PAR0[dx]), (1, PAR1[dx])):
                    nc.sync.dma_start(
                        out=_ap(c_vshift[:], [[2 * vstep, 2], [126, 64], [1, 63]],
                                off=(dx * 4 + wh) * vstep + par * 63),
                        in_=_ap(blob[:], [[8192, 2], [128, 64], [1, 63]],
                                off=OFF_EO + poff),
                    )

        bias1 = _ap(c_bb[:], [[bbstep, 128], [1, 1]], off=0)
        bias2 = _ap(c_bb[:], [[bbstep, 128], [1, 1]], off=1)

        ngroups = (T + 3) // 4
        for g in range(ngroups):
            tok0 = g * 4
            # ---- stage A: V[(dy,co), (t4,par,j2)] = sum_dx w0 * v(shifted) ----
            psV = ps.tile([96, 504], F32, tag="ps")
            nc.tensor.matmul(
                psV[:, :], lhsT=c_w0s[:], rhs=c_vshift[:, g * 504:(g + 1) * 504],
                start=True, stop=True)
            vhi = sb2.tile([96, 504], BF16, tag="vhi")
            vlo = sb2.tile([96, 504], BF16, tag="vlo")
            nc.vector.tensor_copy(out=vhi[:], in_=psV[:, :])
            nc.vector.tensor_sub(vlo[:], psV[:, :], vhi[:])
            # bounce V to DRAM reordered as [t4, dy, co, (par,j2)=126]
            tvhi = dr.tile([4, 3, 32, 126], BF16, tag="tvhi")
            tvlo = dr.tile([4, 3, 32, 126], BF16, tag="tvlo")
            for src, dst in ((vhi, tvhi), (vlo, tvlo)):
                sstep = src[:].ap[0][0]
                for dy in range(3):
                    nc.sync.dma_start(
                        out=_ap(dst[:], [[126, 32], [12096, 4], [1, 126]],
                                off=dy * 4032),
                        in_=_ap(src[:], [[sstep, 32], [1, 504]],
                                off=dy * 32 * sstep),
                    )

            for t4 in range(min(4, T - tok0)):
                t = tok0 + t4
                # ---- V3_4 rhs [13, 4104]: rows Vh*6+uh*3+dy, free co*128+par*64+j2
                v34 = sb3.tile([13, 4104], BF16, tag="v34")
                v34step = v34[:].ap[0][0]
                for vh, tv in ((0, tvhi), (1, tvlo)):
                    for uh in range(2):
                        for par in range(2):
                            nc.sync.dma_start(
                                out=_ap(v34[:], [[v34step, 3], [128, 32], [1, 63]],
                                        off=(vh * 6 + uh * 3) * v34step + par * 64),
                                in_=_ap(tv[:], [[4032, 3], [126, 32], [1, 63]],
                                        off=t4 * 12096 + par * 63),
                            )
                nc.sync.dma_start(out=v34[12:13, 0:4096], in_=c_b0bc[:])

                # ---- stage B: y1 quarters -> gelu -> g1 [128, 4096] bf16
                g1 = sb2.tile([128, 4096], BF16, tag="g1")
                lhsT_e = _ap(c_ushift[:], [[upstep, 13], [2, 64]], off=t * 126)
                lhsT_o = _ap(c_ushift[:], [[upstep, 13], [2, 64]], off=t * 126 + 1)
                for q in range(4):
                    pq = ps.tile([128, 1024], F32, tag="ps")
                    for half in range(2):  # chunk c = 2q+half covers co 4c..4c+3
                        c = 2 * q + half
                        rhs = v34[:, c * 512:(c + 1) * 512]
                        nc.tensor.matmul(pq[0:64, half * 512:(half + 1) * 512],
                                         lhsT=lhsT_e, rhs=rhs, start=True, stop=True,
                                         tile_position=(0, 0))
                        nc.tensor.matmul(pq[64:128, half * 512:(half + 1) * 512],
                                         lhsT=lhsT_o, rhs=rhs, start=True, stop=True,
                                         tile_position=(0, 64))
                    nc.scalar.activation(out=g1[:, q * 1024:(q + 1) * 1024],
                                         in_=pq[:, :], func=GELU)

                # ---- pool1: j-pairs (parity halves) then i-pairs (e/o halves)
                g1step = g1[:].ap[0][0]
                mj = sb2.tile([128, 2048], BF16, tag="mj")
                nc.vector.tensor_max(
                    mj[:],
                    _ap(g1[:], [[g1step, 128], [128, 32], [1, 64]]),
                    _ap(g1[:], [[g1step, 128], [128, 32], [1, 64]], off=64),
                )
                # ---- pool1 i-pairs: shift odd half down to partition 0 via DMA,
                # then max on gpsimd (tensor_tensor needs equal base partitions)
                mjstep = mj[:].ap[0][0]
                mjb = sb2.tile([64, 2048], BF16, tag="mjb")
                nc.sync.dma_start(
                    out=mjb[0:63, :],
                    in_=_ap(mj[:], [[mjstep, 63], [1, 2048]], off=64 * mjstep),
                )
                p1 = sb2.tile([63, 2048], BF16, tag="p1")
                nc.vector.tensor_max(p1[:], mj[0:63, :], mjb[0:63, :])

                # ---- bounce p1 -> tmp1[co, i2*63+j2]
                tmp1 = dr.tile([32, 3969], BF16, tag="tmp1")
                nc.sync.dma_start(
                    out=_ap(tmp1[:], [[63, 63], [3969, 32], [1, 63]]),
                    in_=_ap(p1[:], [[p1[:].ap[0][0], 63], [64, 32], [1, 63]]),
                )
                # ---- S2 [96, 4104]: row (dy,ci)[y*64+x] = tmp1[ci, (y+dy)*63+x]
                s2 = sb3.tile([96, 4104], BF16, tag="s2")
                s2step = s2[:].ap[0][0]
                for dy in range(3):
                    nc.sync.dma_start(
                        out=_ap(s2[:], [[s2step, 32], [64, 63 - dy], [1, 63]],
                                off=dy * 32 * s2step),
                        in_=_ap(tmp1[:], [[3969, 32], [63, 63 - dy], [1, 63]],
                                off=dy * 63),
                    )

                # ---- conv2: 8 chunks x 3 dx, col-tiled pairs, 2 psum slots
                g2 = sb2.tile([128, 2048], BF16, tag="g2")
                for s in range(2):
                    p2s = ps.tile([128, 1024], F32, tag="ps")
                    for k in range(4):
                        c = 4 * s + k
                        pos = 64 * (c % 2)
                        od = 512 * (k // 2)
                        for dx in range(3):
                            nc.tensor.matmul(
                                p2s[pos:pos + 64, od:od + 512],
                                lhsT=c_w1s[:, dx * 64:(dx + 1) * 64],
                                rhs=s2[:, c * 512 + dx: c * 512 + dx + 512],
                                start=(dx == 0), stop=(dx == 2),
                                tile_position=(0, pos))
                    nc.scalar.activation(out=g2[:, s * 1024:(s + 1) * 1024],
                                         in_=p2s[:, :], func=GELU, bias=bias1)

                # ---- pool2: x-pairs then y-pairs
                g2step = g2[:].ap[0][0]
                mx2 = sb2.tile([128, 1024], BF16, tag="mx2")
                mx2step = mx2[:].ap[0][0]
                nc.vector.tensor_max(
                    _ap(mx2[:], [[mx2step, 128], [256, 4], [32, 8], [1, 31]]),
                    _ap(g2[:], [[g2step, 128], [512, 4], [64, 8], [2, 31]]),
                    _ap(g2[:], [[g2step, 128], [512, 4], [64, 8], [2, 31]], off=1),
                )
                p2 = sb2.tile([128, 512], BF16, tag="p2")
                p2step = p2[:].ap[0][0]
                nc.vector.tensor_max(
                    _ap(p2[:], [[p2step, 128], [128, 4], [32, 4], [1, 32]]),
                    _ap(mx2[:], [[mx2step, 128], [256, 4], [64, 4], [1, 32]]),
                    _ap(mx2[:], [[mx2step, 128], [256, 4], [64, 4], [1, 32]], off=32),
                )

                # ---- bounce p2 -> tmp3[co, y2*30+x2] (alloc 1024 with junk pad)
                tmp3 = dr.tile([64, 1024], BF16, tag="tmp3")
                for b in range(4):  # top half: y2 = 8b+q
                    nc.sync.dma_start(
                        out=_ap(tmp3[:], [[1024, 64], [30, 4], [1, 30]],
                                off=b * 240),
                        in_=_ap(p2[:], [[p2step, 64], [32, 4], [1, 30]],
                                off=b * 128),
                    )
                for b in range(4):  # bottom half: y2 = 8b+4+q (b=3: only q<2)
                    nq = 4 if b < 3 else 2
                    nc.sync.dma_start(
                        out=_ap(tmp3[:], [[1024, 64], [30, nq], [1, 30]],
                                off=b * 240 + 120),
                        in_=_ap(p2[:], [[p2step, 64], [32, nq], [1, 30]],
                                off=64 * p2step + b * 128),
                    )

                # ---- S3a [128, 968] rows (dy01, ci); S3b [64, 968] dy=2
                s3a = sb2.tile([128, 968], BF16, tag="s3a")
                s3b = sb2.tile([64, 968], BF16, tag="s3b")
                nc.sync.dma_start(
                    out=s3a[:, 0:962],
                    in_=_ap(tmp3[:], [[30, 2], [1024, 64], [1, 962]]),
                )
                nc.sync.dma_start(
                    out=s3b[:, 0:962],
                    in_=_ap(tmp3[:], [[1024, 64], [1, 962]], off=60),
                )

                # ---- conv3: 2 chunks (N=480) col-tiled, 6 accumulating MMs each
                p3s = ps.tile([128, 1024], F32, tag="ps")
                for c in range(2):
                    pos = 64 * c
                    for dx in range(3):
                        nc.tensor.matmul(
                            p3s[pos:pos + 64, 0:480],
                            lhsT=c_w2a[:, dx * 64:(dx + 1) * 64],
                            rhs=s3a[:, c * 480 + dx: c * 480 + dx + 480],
                            start=(dx == 0), stop=False, tile_position=(0, pos))
                    for dx in range(3):
                        nc.tensor.matmul(
                            p3s[pos:pos + 64, 0:480],
                            lhsT=c_w2b[:, dx * 64:(dx + 1) * 64],
                            rhs=s3b[:, c * 480 + dx: c * 480 + dx + 480],
                            start=False, stop=(dx == 2), tile_position=(0, pos))
                g3 = sb2.tile([128, 480], BF16, tag="g3")
                nc.scalar.activation(out=g3[:], in_=p3s[:, 0:480], func=GELU,
                                     bias=bias2)

                # ---- pool3
                g3step = g3[:].ap[0][0]
                mx3 = sb2.tile([128, 256], BF16, tag="mx3")
                mx3step = mx3[:].ap[0][0]
                nc.vector.tensor_max(
                    _ap(mx3[:], [[mx3step, 128], [16, 16], [1, 15]]),
                    _ap(g3[:], [[g3step, 128], [30, 16], [2, 15]]),
                    _ap(g3[:], [[g3step, 128], [30, 16], [2, 15]], off=1),
                )
                p3 = sb2.tile([128, 128], BF16, tag="p3")
                p3step = p3[:].ap[0][0]
                nc.vector.tensor_max(
                    _ap(p3[:], [[p3step, 128], [16, 8], [1, 16]]),
                    _ap(mx3[:], [[mx3step, 128], [32, 8], [1, 16]]),
                    _ap(mx3[:], [[mx3step, 128], [32, 8], [1, 16]], off=16),
                )

                # ---- output: out[t, co*196 + y2*14 + x2]
                nc.sync.dma_start(
                    out=_ap(out[:], [[196, 64], [14, 8], [1, 14]], off=t * 12544),
                    in_=_ap(p3[:], [[p3step, 64], [16, 8], [1, 14]]),
                )
                nc.sync.dma_start(
                    out=_ap(out[:], [[196, 64], [14, 6], [1, 14]],
                            off=t * 12544 + 112),
                    in_=_ap(p3[:], [[p3step, 64], [16, 6], [1, 14]],
                            off=64 * p3step),
                )

    nc.compile()
    return nc


def _host_prep(token_features, type_embedds, w0, b0, w1, b1, w2, b2):
    """Build per-core packed blob + shared f32 biases."""
    B, S, Dd = token_features.shape
    assert (B, S, Dd) == (8, 64, 128)
    bf = ml_dtypes.bfloat16

    w0h, w0l = _split_bf16(w0.astype(np.float32))
    w0s = np.zeros((12, 96), dtype=bf)
    for dx in range(3):
        for vh in range(2):
            for wh in range(2):
                r = dx * 4 + vh * 2 + wh
                src = (w0h, w0l)[wh][:, 0, :, dx]           # [co, dy]
                w0s[r, :] = np.transpose(src, (1, 0)).reshape(96)  # dy*32+co

    w1b = w1.astype(bf)                                     # [64, 32, 3, 3]
    w1s = np.zeros((96, 192), dtype=bf)
    for dy in range(3):
        for dx in range(3):
            w1s[dy * 32:(dy + 1) * 32, dx * 64:(dx + 1) * 64] = w1b[:, :, dy, dx].T

    w2q = w2.astype(bf)                                     # [64, 64, 3, 3]
    w2a = np.zeros((128, 192), dtype=bf)
    w2b_ = np.zeros((64, 192), dtype=bf)
    for dy in range(2):
        for dx in range(3):
            w2a[dy * 64:(dy + 1) * 64, dx * 64:(dx + 1) * 64] = w2q[:, :, dy, dx].T
    for dx in range(3):
        w2b_[:, dx * 64:(dx + 1) * 64] = w2q[:, :, 2, dx].T

    b0bc = np.repeat(b0.astype(np.float32), 128).astype(bf)  # [4096]
    shared_flat = np.concatenate([w0s.ravel(), w1s.ravel(), w2a.ravel(),
                                  w2b_.ravel(), b0bc])

    bcomb = np.stack([np.concatenate([b1, b1]),
                      np.concatenate([b2, b2])], axis=1).astype(np.float32)

    u = type_embedds.astype(np.float32)                      # [8, 64, 128]
    v = token_features.astype(np.float32)
    uh, ul = _split_bf16(u)
    vh_, vl = _split_bf16(v)
    eoh = np.concatenate([vh_[..., 0::2], vh_[..., 1::2]], axis=-1)  # [8,64,128]
    eol = np.concatenate([vl[..., 0::2], vl[..., 1::2]], axis=-1)

    shared = {"bcomb": bcomb}
    per_core = []
    for c in range(B):
        blob = np.concatenate([
            uh[c].ravel(), ul[c].ravel(),
            eoh[c].ravel(), eol[c].ravel(),
            shared_flat,
        ]).astype(bf).reshape(1, NBLOB)
        per_core.append({"blob": blob})
    return shared, per_core


_CACHE = {}


def _get_kernel(T=64):
    if T not in _CACHE:
        _CACHE[T] = build_kernel(T)
    return _CACHE[T]


def kernel(token_features, type_embedds, w0, b0, w1, b1, w2, b2):
    token_features = np.asarray(token_features, dtype=np.float32)
    type_embedds = np.asarray(type_embedds, dtype=np.float32)
    shared, per_core = _host_prep(token_features, type_embedds,
                                  np.asarray(w0, dtype=np.float32),
                                  np.asarray(b0, dtype=np.float32),
                                  np.asarray(w1, dtype=np.float32),
                                  np.asarray(b1, dtype=np.float32),
                                  np.asarray(w2, dtype=np.float32),
                                  np.asarray(b2, dtype=np.float32))
    nc = _get_kernel(64)
    in_maps = [dict(shared, **per_core[c]) for c in range(NCORES)]
    res = run_bass_kernel_spmd(nc, in_maps, core_ids=list(range(NCORES)))
    outs = np.stack([res.results[c]["out"] for c in range(NCORES)])
    return outs.astype(np.float32)
